# revision 8
# baseline (speedup 1.0000x reference)
"""Max-unpool (DePooling2D) Trainium2 kernel.

Full inputs: net [8,56,56,256] f32, mask [8,56,56,256] int64 (tf argmax
encoding ((y*oW)+x)*C + c with y=2h+dy, x=2w+dx, dy,dx in {0,1}), stride=2.
Output: [8,112,112,256] f32 with net scattered to (2h+dy, 2w+dx, c), zeros
elsewhere.

Strategy: batch dim sharded across the 8 NeuronCores (1 image per core).
On-core, the scatter is recast as a dense select: for input element
(h,w,c) the mask value is 57344*h + 28672*dy + 512*w + 256*dx + c, so
subtracting a per-(partition,c) base leaves t = 114688*it + 28672*dy +
256*dx (h = 2*it + h2; partition p = h2*56 + w).  Each of the four output
positions (i,j) of the 2x2 window is then a single fused DVE op
out_ij = (t == K_ij) * net  via scalar_tensor_tensor.
"""

import numpy as np

import concourse.bass as bass
import concourse.mybir as mybir
from concourse import bacc, bass_utils
from concourse.tile import TileContext

B, H, W, C = 8, 56, 56, 256
OH, OW = 2 * H, 2 * W
NIT = 7           # h-pair iterations per DMA group
NG = (H // 2) // NIT  # 4 groups

_FP = mybir.dt.float32
_I32 = mybir.dt.int32


def _build_bass(two: int) -> bass.Bass:
    """two=2: mask arrives as int64 viewed as int32 pairs (low word first);
    two=1: mask is already int32."""
    nc = bacc.Bacc("TRN2", target_bir_lowering=False, debug=False)
    net = nc.dram_tensor("net", [H, W, C], _FP, kind="ExternalInput").ap()
    mask32 = nc.dram_tensor("mask32", [H, W, two * C], _I32, kind="ExternalInput").ap()
    wc = nc.dram_tensor("wc", [112, C], _FP, kind="ExternalInput").ap()
    out = nc.dram_tensor("out", [OH, OW, C], _FP, kind="ExternalOutput").ap()

    # Partition p = h2*56 + w over input-row pairs; (h2 w) merges to a
    # single stride-256 dim since h2 stride (14336) == 56 * w stride.
    net_r = net.rearrange("(it h2) w c -> (h2 w) it c", h2=2)            # [112,28,256]
    mask_r = mask32.rearrange("(it h2) w (c two) -> (h2 w) it c two", h2=2, two=two)
    # Output rows y = 4*it + 2*h2 + i, columns x = 2*w + j; (j c) is
    # contiguous (512 elems).  h2 kept separate (strides don't merge).
    out_r = out.rearrange("(it h2 i) (w j) c -> h2 w it i (j c)", h2=2, i=2, j=2)

    with TileContext(nc) as tc:
        with (
            tc.tile_pool(name="wcp", bufs=1) as wcp,
            tc.tile_pool(name="inp", bufs=3) as inp,
            tc.tile_pool(name="tp", bufs=4) as tp,
            tc.tile_pool(name="outp", bufs=3) as outp,
        ):
            wct = wcp.tile([112, C], _FP)
            nc.sync.dma_start(out=wct[:], in_=wc)
            for g in range(NG):
                sl = slice(g * NIT, (g + 1) * NIT)
                nett = inp.tile([112, NIT, C], _FP, tag="net")
                maskt = inp.tile([112, NIT, C, two], _I32, tag="mask")
                outt = outp.tile([112, NIT, 2, 2 * C], _FP, tag="out")
                nc.sync.dma_start(out=nett[:], in_=net_r[:, sl, :])
                nc.sync.dma_start(out=maskt[:], in_=mask_r[:, sl, :, :])
                for itl in range(NIT):
                    it = g * NIT + itl
                    tt = tp.tile([112, C], _FP, tag="t")
                    nc.vector.tensor_tensor(
                        out=tt[:],
                        in0=maskt[:, itl, :, 0],
                        in1=wct[:],
                        op=mybir.AluOpType.subtract,
                    )
                    for i in range(2):
                        for j in range(2):
                            k_ij = float(114688 * it + 28672 * i + 256 * j)
                            nc.vector.scalar_tensor_tensor(
                                out=outt[:, itl, i, j * C : (j + 1) * C],
                                in0=tt[:],
                                scalar=k_ij,
                                in1=nett[:, itl, :],
                                op0=mybir.AluOpType.is_equal,
                                op1=mybir.AluOpType.mult,
                            )
                for h2 in range(2):
                    for i in range(2):
                        nc.sync.dma_start(
                            out=out_r[h2, :, sl, i, :],
                            in_=outt[h2 * 56 : (h2 + 1) * 56, :, i, :],
                        )
    nc.compile()
    return nc


_NC_CACHE: dict[int, bass.Bass] = {}


def _get_nc(two: int) -> bass.Bass:
    if two not in _NC_CACHE:
        _NC_CACHE[two] = _build_bass(two)
    return _NC_CACHE[two]


def _make_wc() -> np.ndarray:
    h2 = 57344 * np.arange(2, dtype=np.int64)[:, None, None]
    w = 512 * np.arange(W, dtype=np.int64)[None, :, None]
    c = np.arange(C, dtype=np.int64)[None, None, :]
    return (h2 + w + c).reshape(112, C).astype(np.float32)


def kernel(net: np.ndarray, mask: np.ndarray, stride=None, **run_kwargs):
    net = np.ascontiguousarray(net, dtype=np.float32)
    mask = np.ascontiguousarray(mask)
    assert net.shape == (B, H, W, C) and mask.shape == (B, H, W, C)
    if mask.dtype == np.int64:
        # Little-endian int64 -> low int32 word sits at even indices.
        two = 2
        mask32 = mask.view(np.int32).reshape(B, H, W, 2 * C)
    else:
        two = 1
        mask32 = mask.astype(np.int32, copy=False).reshape(B, H, W, C)
    wc = _make_wc()
    in_maps = [
        {"net": net[k], "mask32": mask32[k], "wc": wc} for k in range(B)
    ]
    nc = _get_nc(two)
    res = bass_utils.run_bass_kernel_spmd(nc, in_maps, list(range(B)), **run_kwargs)
    out = np.stack([res.results[k]["out"] for k in range(B)], axis=0)
    if run_kwargs:
        kernel.last_results = res
    return out
